# revision 5
# baseline (speedup 1.0000x reference)
"""Trainium2 Bass kernel for nn_LlamaAttention (B=1, S=2048, D=4096, H=32, KVH=8, HD=128).

Sharding (8 cores): tensor-parallel over heads. Core c owns Q heads 4c..4c+3 and
KV head c (GQA groups stay intact). Each core projects Q/K/V for its heads in a
TRANSPOSED activation layout ([head_dim, seq], head_dim on partitions), applies
RoPE via host-precomputed cos/sin tables, computes causal attention with a
transposed no-max softmax (denominators via ones-vector matmuls on the PE), then
the per-core head outputs [512, 2048] are AllGathered into the full transposed
attention output [4096, 2048]. Wo is column-parallel: core c computes output
columns [512c, 512c+512) and the host concatenates the 8 column slices.

Matmuls run in float32r (FP22 on the PE, 1 cycle/row at free-dim >= 256), which
keeps ~1e-3-level accuracy at bf16-class throughput.
"""

import math

import numpy as np

# Problem constants (hardcoded per the harness contract).
S = 2048
D = 4096
H = 32
KVH = 8
HD = 128
ROT = 64
HALF = 32
THETA = 10000.0
NCORES = 8
QH = H // NCORES  # 4 query heads per core
P = 128
CH = 512  # seq chunk (matmul moving free dim)
NCH = S // CH  # 4
DT = D // P  # 32 contraction tiles for the projections
KT = S // P  # 16 key tiles

_CACHE = {}


def _build_nc():
    import concourse.mybir as mybir
    from concourse import bacc
    from concourse.bass import ds
    from concourse.masks import make_identity
    from concourse.tile import TileContext

    f32 = mybir.dt.float32
    f32r = mybir.dt.float32r
    EXP = mybir.ActivationFunctionType.Exp

    nc = bacc.Bacc()

    xT = nc.dram_tensor("xT", [D, S], f32r, kind="ExternalInput")
    maskd = nc.dram_tensor("maskd", [KT * P, CH], f32, kind="ExternalInput")
    costab = nc.dram_tensor("costab", [ROT, S], f32, kind="ExternalInput")
    sintab = nc.dram_tensor("sintab", [ROT, S], f32, kind="ExternalInput")
    wq = nc.dram_tensor("wq", [D, QH * HD], f32r, kind="ExternalInput")
    wk = nc.dram_tensor("wk", [D, HD], f32r, kind="ExternalInput")
    wv = nc.dram_tensor("wv", [D, HD], f32r, kind="ExternalInput")
    wo = nc.dram_tensor("wo", [H * HD, CH], f32r, kind="ExternalInput")
    onesd = nc.dram_tensor("onesd", [P, 1], f32r, kind="ExternalInput")
    out = nc.dram_tensor("out", [S, CH], f32, kind="ExternalOutput")
    aout = nc.dram_tensor("aout", [QH * HD, S], f32r)
    aout_all = nc.dram_tensor("aout_all", [H * HD, S], f32r, addr_space="Shared")

    with TileContext(nc) as tc:
        with tc.tile_pool(name="ptab", bufs=1) as ptab, \
             tc.tile_pool(name="pqkv", bufs=1) as pqkv:
            costab_sb = ptab.tile([ROT, S], f32)
            nc.sync.dma_start(costab_sb[:], costab[:])
            sintab_sb = ptab.tile([ROT, S], f32)
            nc.sync.dma_start(sintab_sb[:], sintab[:])
            ones_sb = ptab.tile([P, 1], f32r)
            nc.sync.dma_start(ones_sb[:], onesd[:])
            ident_sb = ptab.tile([P, P], f32)
            make_identity(nc, ident_sb[:])

            qt_sb = pqkv.tile([P, QH, S], f32r)   # Q^T per head (roped, pre-scaled)
            kt_sb = pqkv.tile([P, S], f32r)       # K^T (roped)
            v_sb = pqkv.tile([P, KT, HD], f32r)   # V in natural [sk, hd] tiles
            aout_sb = pqkv.tile([P, QH, S], f32r)  # normalized attention out^T

            def rope(dst, src_psum, sq, prt):
                # dst[:128] <- src; then dst[0:64] = src[0:64]*cos + swap(src)[0:64]*sin_signed
                nc.scalar.copy(dst, src_psum)
                rt = prt.tile([ROT, CH], f32, tag="rt")
                nc.sync.dma_start(rt[0:HALF], dst[HALF:ROT].bitcast(f32))
                nc.sync.dma_start(rt[HALF:ROT], dst[0:HALF].bitcast(f32))
                nc.vector.tensor_mul(dst[0:ROT], dst[0:ROT], costab_sb[:, sq])
                nc.vector.tensor_mul(rt[:], rt[:], sintab_sb[:, sq])
                nc.vector.tensor_add(dst[0:ROT], dst[0:ROT], rt[:])

            # ---------------- Phase 1: QKV projections (transposed) ----------------
            with tc.tile_pool(name="pw1", bufs=1) as pw1, \
                 tc.tile_pool(name="pxt", bufs=3) as pxt, \
                 tc.tile_pool(name="pvt", bufs=2) as pvt, \
                 tc.tile_pool(name="prt", bufs=2) as prt, \
                 tc.tile_pool(name="psq", bufs=4, space="PSUM") as psq_pool, \
                 tc.tile_pool(name="psk", bufs=1, space="PSUM") as psk_pool, \
                 tc.tile_pool(name="psv", bufs=1, space="PSUM") as psv_pool, \
                 tc.tile_pool(name="pst", bufs=2, space="PSUM") as pst_pool:
                wq_sb = pw1.tile([P, DT, QH * HD], f32r)
                nc.sync.dma_start(wq_sb[:], wq.rearrange("(kt p) m -> p kt m", p=P))
                wk_sb = pw1.tile([P, DT, HD], f32r)
                nc.sync.dma_start(wk_sb[:], wk.rearrange("(kt p) m -> p kt m", p=P))
                wv_sb = pw1.tile([P, DT, HD], f32r)
                nc.sync.dma_start(wv_sb[:], wv.rearrange("(kt p) m -> p kt m", p=P))

                for c in range(NCH):
                    sq = ds(c * CH, CH)
                    psq = [psq_pool.tile([P, CH], f32, tag="psq", name=f"psq{_h}") for _h in range(QH)]
                    psk = psk_pool.tile([P, CH], f32, tag="psk")
                    psv = psv_pool.tile([P, CH], f32, tag="psv")
                    for kt in range(DT):
                        xt = pxt.tile([P, CH], f32r, tag="xt")
                        nc.sync.dma_start(xt[:], xT[ds(kt * P, P), sq])
                        xr = xt[:]
                        st = dict(start=(kt == 0), stop=(kt == DT - 1))
                        for h in range(QH):
                            nc.tensor.matmul(
                                psq[h][:], wq_sb[:, kt, ds(h * HD, HD)],
                                xr, **st)
                        nc.tensor.matmul(psk[:], wk_sb[:, kt], xr, **st)
                        nc.tensor.matmul(psv[:], wv_sb[:, kt], xr, **st)
                    for h in range(QH):
                        rope(qt_sb[:, h, sq], psq[h][:], sq, prt)
                    rope(kt_sb[:, sq], psk[:], sq, prt)
                    # V^T chunk -> natural-layout V tiles via PE transpose
                    vt = pvt.tile([P, CH], f32, tag="vt")
                    nc.scalar.copy(vt[:], psv[:])
                    for j in range(4):
                        pst = pst_pool.tile([P, P], f32, tag="pst")
                        nc.tensor.transpose(pst[:], vt[:, ds(j * P, P)], ident_sb[:])
                        nc.vector.tensor_copy(v_sb[:, 4 * c + j], pst[:])

            # ---------------- Phase 2: causal attention ----------------
            with tc.tile_pool(name="pes", bufs=4) as pes, \
                 tc.tile_pool(name="pms", bufs=5) as pms, \
                 tc.tile_pool(name="prb", bufs=2) as prb, \
                 tc.tile_pool(name="pss", bufs=2, space="PSUM") as pss_pool, \
                 tc.tile_pool(name="psd", bufs=2, space="PSUM") as psd_pool, \
                 tc.tile_pool(name="pso", bufs=2, space="PSUM") as pso_pool:
                for c in range(NCH):
                    sq = ds(c * CH, CH)
                    ntile = 4 * c + 4
                    mstiles = {}
                    for t in range(4 * c, ntile):
                        ms = pms.tile([P, CH], f32, tag="ms")
                        nc.sync.dma_start(ms[:], maskd[ds(t * P, P), :])
                        mstiles[t] = ms
                    for h in range(QH):
                        qr = qt_sb[:, h, sq]
                        pso = pso_pool.tile([P, CH], f32, tag="pso")
                        psd = psd_pool.tile([1, CH], f32, tag="psd")
                        for t in range(ntile):
                            pss = pss_pool.tile([P, CH], f32, tag="pss")
                            nc.tensor.matmul(
                                pss[:], kt_sb[:, ds(t * P, P)], qr,
                                start=True, stop=True)
                            es = pes.tile([P, CH], f32r, tag="es")
                            if t >= 4 * c:
                                nc.vector.tensor_add(es[:], pss[:], mstiles[t][:])
                                nc.scalar.activation(es[:], es[:], EXP)
                            else:
                                nc.scalar.activation(es[:], pss[:], EXP)
                            esr = es[:]
                            st = dict(start=(t == 0), stop=(t == ntile - 1))
                            nc.tensor.matmul(psd[:], ones_sb[:], esr, **st)
                            nc.tensor.matmul(pso[:], v_sb[:, t], esr, **st)
                        rcp = prb.tile([1, CH], f32, tag="rcp")
                        nc.vector.reciprocal(rcp[:], psd[:])
                        rb = prb.tile([P, CH], f32, tag="rb")
                        nc.gpsimd.partition_broadcast(rb[:], rcp[:])
                        nc.vector.tensor_mul(aout_sb[:, h, sq], pso[:], rb[:])

            for h in range(QH):
                nc.sync.dma_start(aout[ds(h * HD, HD), :], aout_sb[:, h, :])
            nc.gpsimd.collective_compute(
                "AllGather",
                mybir.AluOpType.bypass,
                ins=[aout[:]],
                outs=[aout_all[:]],
                replica_groups=[list(range(NCORES))],
            )

            # ---------------- Phase 3: output projection (column slice) ----------------
            with tc.tile_pool(name="pwo", bufs=1) as pwo, \
                 tc.tile_pool(name="pat", bufs=3) as pat, \
                 tc.tile_pool(name="pob", bufs=3) as pob, \
                 tc.tile_pool(name="psw", bufs=8, space="PSUM") as psw_pool:
                wo_sb = pwo.tile([P, DT, CH], f32r)
                wo_r = wo.rearrange("(kt p) m -> p kt m", p=P)
                for g in range(8):
                    nc.sync.dma_start(wo_sb[:, ds(g * 4, 4)], wo_r[:, ds(g * 4, 4)])
                for ss in range(S // CH):
                    psw = [psw_pool.tile([P, CH], f32, tag="psw", name=f"psw{_j}") for _j in range(4)]
                    for kt in range(DT):
                        at = pat.tile([P, CH], f32r, tag="at")
                        nc.sync.dma_start(
                            at[:], aout_all[ds(kt * P, P), ds(ss * CH, CH)])
                        wr = wo_sb[:, kt]
                        st = dict(start=(kt == 0), stop=(kt == DT - 1))
                        for j in range(4):
                            nc.tensor.matmul(
                                psw[j][:], at[:, ds(j * P, P)], wr, **st)
                    for j in range(4):
                        ob = pob.tile([P, CH], f32, tag="ob")
                        nc.vector.tensor_copy(ob[:], psw[j][:])
                        nc.sync.dma_start(out[ds(ss * CH + j * P, P), :], ob[:])

    nc.finalize()
    return nc


def _get_nc():
    if "nc" not in _CACHE:
        _CACHE["nc"] = _build_nc()
    return _CACHE["nc"]


def _host_prep(hidden_states, attention_mask, position_ids, Wq, Wk, Wv, Wo):
    hidden_states = np.asarray(hidden_states, dtype=np.float32)
    attention_mask = np.asarray(attention_mask, dtype=np.float32)
    position_ids = np.asarray(position_ids)
    Wq = np.asarray(Wq, dtype=np.float32)
    Wk = np.asarray(Wk, dtype=np.float32)
    Wv = np.asarray(Wv, dtype=np.float32)
    Wo = np.asarray(Wo, dtype=np.float32)

    x = hidden_states.reshape(S, D)
    mask = attention_mask.reshape(S, S)
    pos = position_ids.reshape(S).astype(np.float32)

    xT = np.ascontiguousarray(x.T)
    # diagonal mask tiles, transposed: maskd[128t:128(t+1)] = mask[sq-chunk, sk-tile].T
    maskd = np.concatenate(
        [np.ascontiguousarray(
            mask[(t // 4) * CH:(t // 4 + 1) * CH, t * P:(t + 1) * P].T)
         for t in range(KT)], axis=0)

    freqs = (1.0 / THETA ** (np.arange(0, HD, 2, dtype=np.float32) / HD)).astype(np.float32)
    ang = pos[:, None] * freqs[None, :]
    costab = np.ascontiguousarray(np.cos(ang).T)
    sint = np.sin(ang).T
    sintab = np.ascontiguousarray(np.concatenate([-sint[:HALF], sint[HALF:]], axis=0))

    scale = np.float32(1.0 / math.sqrt(HD))
    in_maps = []
    for c in range(NCORES):
        in_maps.append({
            "xT": xT,
            "onesd": np.ones((P, 1), np.float32),
            "maskd": maskd,
            "costab": costab,
            "sintab": sintab,
            "wq": np.ascontiguousarray(Wq[:, c * QH * HD:(c + 1) * QH * HD]) * scale,
            "wk": np.ascontiguousarray(Wk[:, c * HD:(c + 1) * HD]),
            "wv": np.ascontiguousarray(Wv[:, c * HD:(c + 1) * HD]),
            "wo": np.ascontiguousarray(Wo[:, c * CH:(c + 1) * CH]),
        })
    return in_maps


def _run(inputs, trace=False):
    from concourse.bass_utils import run_bass_kernel_spmd

    if trace:
        # NTFF profiling needs antenv.axon_hooks; provide it if the image lacks it.
        try:
            import antenv.axon_hooks  # noqa: F401
        except ImportError:
            import sys
            import types
            try:
                import trn_agent_boot.trn_boot as _tb
                _hook = _tb._ntff_profile_via_ctypes("/opt/axon/libaxon_pjrt.so")
                _m = types.ModuleType("antenv.axon_hooks")
                _m.get_axon_ntff_profile_hook = lambda: _hook
                _m.set_axon_ntff_profile_hook = lambda h: None
                sys.modules["antenv.axon_hooks"] = _m
            except Exception:
                trace = False

    nc = _get_nc()
    in_maps = _host_prep(**inputs)
    res = run_bass_kernel_spmd(nc, in_maps, core_ids=list(range(NCORES)), trace=trace)
    full = np.concatenate(
        [res.results[c]["out"] for c in range(NCORES)], axis=1)[None]
    return np.ascontiguousarray(full, dtype=np.float32), res


def kernel(hidden_states, attention_mask, position_ids, Wq, Wk, Wv, Wo):
    out, _ = _run(dict(
        hidden_states=hidden_states, attention_mask=attention_mask,
        position_ids=position_ids, Wq=Wq, Wk=Wk, Wv=Wv, Wo=Wo))
    return out
